# revision 9
# baseline (speedup 1.0000x reference)
"""Trainium2 Bass kernel for nn_AACls_87282325389858 (moe_routing).

Data-parallel over batch B=8 across 8 NeuronCores (one batch element per
core, no collectives).  Per-core network:

  xn  = LN1(x)                                      [1024, 768]
  g   = concat-softmaxes(xn @ {wg,wg0,wg1}^T)  ->  per-head gates mg [1024, 8]
        (top_k(gates, 6) over 6 routed gates is the identity -> mask == 1)
  qkv = xn @ qkv_w^T ; per head: attn = softmax(q k^T / sqrt(96))  (output!)
  o_h = (mg_h / rowsum_h) * exp(scores_h) @ v_h   -- gate folded into an
        augmented contraction row:  exp(s + log(mg/rowsum))
  a   = concat_h(o_h) @ proj_w^T + proj_b ;  h = 2*a
  out = h + MLP(LN2(h))          ;  MLP = gelu(. @ fc1^T + b) @ fc2^T + b

Returns (out [8,1024,768] f32, attn [8,8,1024,1024] f32) matching the
reference tuple.
"""
import sys

sys.path.insert(0, "/opt/trn_rl_repo")

import numpy as np
import ml_dtypes

import concourse.bass as bass
import concourse.mybir as mybir
from concourse import bacc
from concourse.tile import TileContext
from concourse.masks import make_identity

F32 = mybir.dt.float32
F32R = mybir.dt.float32r
BF16 = mybir.dt.bfloat16
AF = mybir.ActivationFunctionType
OP = mybir.AluOpType
AX = mybir.AxisListType

B, N, C, H, D, HID = 8, 1024, 768, 8, 96, 3072
NCH = 8          # token chunks of 128
CCH = 6          # feature chunks of 128 (C = 768)
HCH = 24         # hidden chunks of 128 (HID = 3072)
SCALE = float(D) ** -0.5
EPS = 1e-5
NQ = 4           # n-quarters for the MLP (256 tokens each)

BF = ml_dtypes.bfloat16


def _bf(a):
    return np.ascontiguousarray(a).astype(BF)


def _f32(a):
    return np.ascontiguousarray(np.asarray(a, dtype=np.float32))


def build():
    nc = bacc.Bacc()

    # ---------------- parameters (per-core shapes) ----------------
    x_p = nc.declare_dram_parameter("x", [128, NCH, C], F32, isOutput=False)
    qkw_p = nc.declare_dram_parameter("qkw_t", [128, CCH, 2 * H, D], BF16, isOutput=False)
    vw_p = nc.declare_dram_parameter("vw_t", [128, CCH, H * D], BF16, isOutput=False)
    gw_p = nc.declare_dram_parameter("gw_t", [128, CCH, 10], BF16, isOutput=False)
    pjw_p = nc.declare_dram_parameter("projw_t", [D, H, C], BF16, isOutput=False)
    f1w_p = nc.declare_dram_parameter("fc1w_t", [128, CCH, HID], BF16, isOutput=False)
    f2w_p = nc.declare_dram_parameter("fc2w_t", [128, HCH, C], BF16, isOutput=False)
    pw1_p = nc.declare_dram_parameter("pw1", [128, CCH], F32, isOutput=False)
    pb1_p = nc.declare_dram_parameter("pb1", [128, CCH], F32, isOutput=False)
    pw2_p = nc.declare_dram_parameter("pw2", [128, CCH], F32, isOutput=False)
    pb2_p = nc.declare_dram_parameter("pb2", [128, CCH], F32, isOutput=False)
    pjb_p = nc.declare_dram_parameter("projb2", [128, CCH], F32, isOutput=False)
    f1b_p = nc.declare_dram_parameter("fc1b", [128, HCH], F32, isOutput=False)
    f2b_p = nc.declare_dram_parameter("fc2b", [128, CCH], F32, isOutput=False)
    ones_p = nc.declare_dram_parameter("ones_row", [1, H, N], F32R, isOutput=False)

    out_p = nc.declare_dram_parameter("out", [N, C], F32, isOutput=True)
    attn_p = nc.declare_dram_parameter("attn", [H, N, N], F32, isOutput=True)

    with TileContext(nc) as tc:
        p0 = tc.alloc_tile_pool(name="const", bufs=1)
        pt = tc.alloc_tile_pool(name="tiny", bufs=6)
        pmm = tc.alloc_tile_pool(name="pmm", bufs=6, space="PSUM")
        pacc = tc.alloc_tile_pool(name="pacc", bufs=2, space="PSUM")

        # ---------------- constants ----------------
        for cv in (EPS, 1.0 / C):
            ct = nc.alloc_sbuf_tensor(f"constap-{cv}", [128, 1], F32)
            nc.gpsimd.memset(ct.ap(), cv)
            nc.const_aps.aps[(F32, cv)] = ct.ap()

        ident_f = p0.tile([128, 128], F32, tag="identf")
        make_identity(nc, ident_f)
        ident_r = p0.tile([128, 128], F32R, tag="identr")
        nc.vector.tensor_copy(ident_r[:], ident_f[:])
        ident_b = p0.tile([128, 128], BF16, tag="identb")
        nc.vector.tensor_copy(ident_b[:], ident_f[:])
        ones_f = p0.tile([128, 1], F32, tag="onesf")
        nc.vector.memset(ones_f[:], 1.0)
        ones_r = p0.tile([128, 1], F32R, tag="onesr")
        nc.vector.tensor_copy(ones_r[:], ones_f[:])
        ones_b = p0.tile([128, 1], BF16, tag="onesb")
        nc.vector.tensor_copy(ones_b[:], ones_f[:])

        pw1 = p0.tile([128, CCH], F32, tag="pw1")
        pb1 = p0.tile([128, CCH], F32, tag="pb1")
        pw2 = p0.tile([128, CCH], F32, tag="pw2")
        pb2 = p0.tile([128, CCH], F32, tag="pb2")
        pjb = p0.tile([128, CCH], F32, tag="pjb")
        f1b = p0.tile([128, HCH], F32, tag="f1b")
        f2b = p0.tile([128, CCH], F32, tag="f2b")
        for t, p in ((pw1, pw1_p), (pb1, pb1_p), (pw2, pw2_p), (pb2, pb2_p),
                     (pjb, pjb_p), (f1b, f1b_p), (f2b, f2b_p)):
            nc.sync.dma_start(out=t[:], in_=p[:])
        gw = p0.tile([128, CCH, 10], BF16, tag="gw")
        nc.sync.dma_start(out=gw[:], in_=gw_p[:])

        # gate / softmax working state (token-partition layout)
        musum = p0.tile([128, NCH], F32, tag="musum")
        sqsum = p0.tile([128, NCH], F32, tag="sqsum")
        mean_g = p0.tile([128, NCH], F32, tag="meang")
        var_g = p0.tile([128, NCH], F32, tag="varg")
        istd_g = p0.tile([128, NCH], F32, tag="istdg")
        ge = p0.tile([128, NCH, 10], F32, tag="ge")
        gsum = p0.tile([128, 3 * NCH], F32, tag="gsum")
        ginv = p0.tile([128, 3 * NCH], F32, tag="ginv")
        mg = p0.tile([128, NCH, H], F32, tag="mg")
        rs2 = p0.tile([128, NCH, H, 2], F32, tag="rs2")
        rs = p0.tile([128, NCH, H], F32, tag="rs")
        invr = p0.tile([128, NCH], F32, tag="invr")
        fct = p0.tile([128, NCH], F32, tag="fct")
        lgf = p0.tile([128, NCH], F32R, tag="lgf")

        # ================= phase A: LN1 + transposes + QKV + gates =========
        pA = tc.alloc_tile_pool(name="pA", bufs=1, side="right")
        x_sb = pA.tile([128, NCH, C], F32, tag="x")
        nc.sync.dma_start(out=x_sb[:], in_=x_p[:])
        xn_bf = pA.tile([128, NCH, C], BF16, tag="xn")
        xnT = pA.tile([128, CCH, N], BF16, tag="xnT")
        qkw = pA.tile([128, CCH, 2 * H, D], BF16, tag="qkw")
        vw = pA.tile([128, CCH, H * D], BF16, tag="vw")
        nc.sync.dma_start(out=qkw[:], in_=qkw_p[:])
        nc.sync.dma_start(out=vw[:], in_=vw_p[:])

        # LN1 statistics
        for ncx in range(NCH):
            nc.vector.reduce_sum(musum[:, ncx : ncx + 1], x_sb[:, ncx, :], axis=AX.X)
            scr = pA.tile([128, C], F32, tag="sqscr")
            nc.scalar.activation(scr[:], x_sb[:, ncx, :], AF.Square,
                                 accum_out=sqsum[:, ncx : ncx + 1])
        nc.vector.tensor_scalar(mean_g[:], musum[:], 1.0 / C, None, OP.mult)
        nc.vector.tensor_scalar(var_g[:], sqsum[:], 1.0 / C, None, OP.mult)
        m2t = pt.tile([128, NCH], F32, tag="m2")
        nc.vector.tensor_mul(m2t[:], mean_g[:], mean_g[:])
        nc.vector.tensor_sub(var_g[:], var_g[:], m2t[:])
        nc.scalar.activation(var_g[:], var_g[:], AF.Sqrt, bias=EPS)
        nc.vector.reciprocal(istd_g[:], var_g[:])
        for ncx in range(NCH):
            nc.vector.tensor_scalar(xn_bf[:, ncx, :], x_sb[:, ncx, :],
                                    mean_g[:, ncx : ncx + 1], istd_g[:, ncx : ncx + 1],
                                    OP.subtract, OP.mult)
        # transpose xn -> xnT (bf16), applying norm1 w/b per c-partition
        for ncx in range(NCH):
            for cc in range(CCH):
                tp = pmm.tile([128, 128], BF16, tag="mm")
                nc.tensor.transpose(tp[:], xn_bf[:, ncx, 128 * cc : 128 * (cc + 1)], ident_b[:])
                nc.vector.tensor_scalar(xnT[:, cc, 128 * ncx : 128 * (ncx + 1)], tp[:],
                                        pw1[:, cc : cc + 1], pb1[:, cc : cc + 1],
                                        OP.mult, OP.add)

        # ---- phase B tensors (left side; allocated before A dies) ----
        pLg = tc.alloc_tile_pool(name="pLg", bufs=2)
        pB1 = tc.alloc_tile_pool(name="pB1", bufs=1)
        pB2 = tc.alloc_tile_pool(name="pB2", bufs=3)
        qT = pB1.tile([D + 1, H, N], F32R, tag="qT")
        kT = pB1.tile([D + 1, H, N], F32R, tag="kT")
        v_sb = pB1.tile([128, NCH, H, D], BF16, tag="v")

        # q/k projections: psum[d, n] = sum_c qkw[c, i, d] * xnT[c, n]
        for i in range(2 * H):  # 0..7 = q heads (pre-scaled), 8..15 = k heads
            dst = qT if i < H else kT
            h = i % H
            for half in range(2):
                ps = pacc.tile([D, 512], F32, tag="acc")
                for cc in range(CCH):
                    nc.tensor.matmul(ps[:], qkw[:, cc, i, :],
                                     xnT[:, cc, 512 * half : 512 * (half + 1)],
                                     start=(cc == 0), stop=(cc == CCH - 1))
                nc.scalar.activation(dst[0:D, h, 512 * half : 512 * (half + 1)], ps[:], AF.Copy)
        # ones row of kT (from host constant)
        nc.sync.dma_start(out=kT[D : D + 1, :, :], in_=ones_p[:])

        # v projection: psum[n, hd] = sum_c xnT[c, n] * vw[c, hd]
        for ncx in range(NCH):
            for half, (lo, w) in enumerate(((0, 512), (512, 256))):
                ps = pacc.tile([128, 512], F32, tag="acc")
                for cc in range(CCH):
                    nc.tensor.matmul(ps[:, :w], xnT[:, cc, 128 * ncx : 128 * (ncx + 1)],
                                     vw[:, cc, lo : lo + w],
                                     start=(cc == 0), stop=(cc == CCH - 1))
                nc.vector.tensor_copy(v_sb[:, ncx, :, :].rearrange("p h d -> p (h d)")[:, lo : lo + w],
                                      ps[:, :w])

        # gates: logits [n, 10] -> exp -> partial sums
        for ncx in range(NCH):
            ps = pmm.tile([128, 10], F32, tag="mm")
            for cc in range(CCH):
                nc.tensor.matmul(ps[:], xnT[:, cc, 128 * ncx : 128 * (ncx + 1)], gw[:, cc, :],
                                 start=(cc == 0), stop=(cc == CCH - 1))
            nc.scalar.activation(ge[:, ncx, :], ps[:], AF.Exp)
            nc.vector.reduce_sum(gsum[:, 3 * ncx : 3 * ncx + 1], ge[:, ncx, 0:6], axis=AX.X)
            nc.vector.reduce_sum(gsum[:, 3 * ncx + 1 : 3 * ncx + 2], ge[:, ncx, 6:8], axis=AX.X)
            nc.vector.reduce_sum(gsum[:, 3 * ncx + 2 : 3 * ncx + 3], ge[:, ncx, 8:10], axis=AX.X)
        nc.vector.reciprocal(ginv[:], gsum[:])
        # mg[:, :, 0:2] = 4 * e8..9 * e6 / (s0 * s1);  mg[:, :, 2:8] = 12 * e0..5 * e7 / (s0 * sr)
        for ncx in range(NCH):
            ta = pt.tile([128, 1], F32, tag="ta")
            tb = pt.tile([128, 1], F32, tag="tb")
            nc.vector.tensor_mul(ta[:], ge[:, ncx, 6:7], ginv[:, 3 * ncx + 1 : 3 * ncx + 2])
            nc.vector.tensor_mul(ta[:], ta[:], ginv[:, 3 * ncx + 2 : 3 * ncx + 3])
            nc.vector.tensor_scalar(ta[:], ta[:], 4.0, None, OP.mult)
            nc.vector.tensor_mul(tb[:], ge[:, ncx, 7:8], ginv[:, 3 * ncx + 1 : 3 * ncx + 2])
            nc.vector.tensor_mul(tb[:], tb[:], ginv[:, 3 * ncx : 3 * ncx + 1])
            nc.vector.tensor_scalar(tb[:], tb[:], 12.0, None, OP.mult)
            nc.vector.tensor_scalar(mg[:, ncx, 0:2], ge[:, ncx, 8:10], ta[:], None, OP.mult)
            nc.vector.tensor_scalar(mg[:, ncx, 2:8], ge[:, ncx, 0:6], tb[:], None, OP.mult)

        pA.release()

        # ================= phase B: attention heads ========================
        pC2 = tc.alloc_tile_pool(name="pC2", bufs=1, side="right")
        pC1 = tc.alloc_tile_pool(name="pC1", bufs=1, side="right")
        hT = pC2.tile([128, CCH, N], F32R, tag="hT")
        o_gT = pC1.tile([D, H, N], BF16, tag="ogT")
        pjw = pC1.tile([D, H, C], BF16, tag="pjw")
        nc.sync.dma_start(out=pjw[:], in_=pjw_p[:])

        def natural(h):
            """scores -> exp -> rowsums -> normalized attn -> HBM"""
            for ncx in range(NCH):
                exp_sb = pB2.tile([128, N], F32, tag="exp")
                for half in range(2):
                    ps = pmm.tile([128, 512], F32, tag="mm")
                    nc.tensor.matmul(ps[:], qT[0:D, h, 128 * ncx : 128 * (ncx + 1)],
                                     kT[0:D, h, 512 * half : 512 * (half + 1)],
                                     start=True, stop=True)
                    nc.scalar.activation(exp_sb[:, 512 * half : 512 * (half + 1)], ps[:],
                                         AF.Exp, accum_out=rs2[:, ncx, h, half : half + 1])
                nc.vector.tensor_add(rs[:, ncx, h : h + 1], rs2[:, ncx, h, 0:1],
                                     rs2[:, ncx, h, 1:2])
                ti = pt.tile([128, 1], F32, tag="ti")
                nc.vector.reciprocal(ti[:], rs[:, ncx, h : h + 1])
                nc.vector.tensor_scalar(exp_sb[:], exp_sb[:], ti[:], None, OP.mult)
                nc.sync.dma_start(out=attn_p[h, 128 * ncx : 128 * (ncx + 1), :], in_=exp_sb[:])

        def transposed(h):
            """log-factor row -> scoresT + gate row -> exp -> o_gT = v^T @ expT"""
            nc.vector.reciprocal(invr[:], rs[:, :, h])
            nc.vector.tensor_mul(fct[:], mg[:, :, h], invr[:])
            nc.scalar.activation(lgf[:], fct[:], AF.Ln)
            lgT = pLg.tile([1, N], F32R, tag="lgT")
            for half in range(2):
                lps = pmm.tile([1, 512], F32R, tag="mm")
                for j in range(4):
                    ncx = 4 * half + j
                    nc.tensor.transpose(lps[:, 128 * j : 128 * (j + 1)],
                                        lgf[:, ncx : ncx + 1], ident_r[:])
                nc.scalar.activation(lgT[:, 512 * half : 512 * (half + 1)], lps[:], AF.Copy)
            nc.sync.dma_start(out=qT[D : D + 1, h, :], in_=lgT[:])

            oT = [pacc.tile([D, 512], F32, tag="acc", name=f"oT{h}_{j}") for j in range(2)]
            for mc in range(NCH):
                expT = pB2.tile([128, N], BF16, tag="expT")
                for half in range(2):
                    ps = pmm.tile([128, 512], F32, tag="mm")
                    nc.tensor.matmul(ps[:], kT[:, h, 128 * mc : 128 * (mc + 1)],
                                     qT[:, h, 512 * half : 512 * (half + 1)],
                                     start=True, stop=True)
                    nc.scalar.activation(expT[:, 512 * half : 512 * (half + 1)], ps[:], AF.Exp)
                for half in range(2):
                    nc.tensor.matmul(oT[half][:], v_sb[:, mc, h, :],
                                     expT[:, 512 * half : 512 * (half + 1)],
                                     start=(mc == 0), stop=(mc == NCH - 1))
            for half in range(2):
                nc.scalar.activation(o_gT[:, h, 512 * half : 512 * (half + 1)],
                                     oT[half][:], AF.Copy)

        natural(0)
        for h in range(H):
            if h + 1 < H:
                natural(h + 1)
            transposed(h)

        pB2.release()
        pB1.release()
        pLg.release()

        # ================= proj + residual (hT = 2*(a + proj_b)) ==========
        pD1 = tc.alloc_tile_pool(name="pD1", bufs=1)
        f1w = pD1.tile([128, CCH, HID], BF16, tag="f1w")
        f2w = pD1.tile([128, HCH, C], BF16, tag="f2w")
        nc.sync.dma_start(out=f1w[:], in_=f1w_p[:])
        nc.sync.dma_start(out=f2w[:], in_=f2w_p[:])

        for cc in range(CCH):
            for half in range(2):
                ps = pacc.tile([128, 512], F32, tag="acc")
                for h in range(H):
                    nc.tensor.matmul(ps[:], pjw[:, h, 128 * cc : 128 * (cc + 1)],
                                     o_gT[:, h, 512 * half : 512 * (half + 1)],
                                     start=(h == 0), stop=(h == H - 1))
                nc.vector.tensor_scalar(hT[:, cc, 512 * half : 512 * (half + 1)], ps[:],
                                        2.0, pjb[:, cc : cc + 1], OP.mult, OP.add)
        pC1.release()

        # ================= LN2 ============================================
        pD2 = tc.alloc_tile_pool(name="pD2", bufs=1)
        pD2b = tc.alloc_tile_pool(name="pD2b", bufs=2)
        hsq = pD2.tile([128, CCH, N], BF16, tag="hsq_h1")  # 12KB/part, slot shared with h1
        hnT = pD2.tile([128, CCH, N], BF16, tag="hnT")
        
        outT = pD2.tile([128, CCH, N], F32R, tag="outT")
        meanb = pD2.tile([128, N], F32, tag="meanb")
        istdb = pD2.tile([128, N], F32, tag="istdb")
        mean_s = pD2.tile([1, N], F32, tag="means")
        sqm_s = pD2.tile([1, N], F32, tag="sqms")
        m2_s = pD2.tile([1, N], F32, tag="m2s")

        for cc in range(CCH):
            nc.scalar.activation(hsq[:, cc, :], hT[:, cc, :], AF.Square)
        mu_ps = [pmm.tile([1, 512], F32, tag="mm", name=f"mups{j}") for j in range(2)]
        sq_ps = [pmm.tile([1, 512], F32, tag="mm", name=f"sqps{j}") for j in range(2)]
        for half in range(2):
            for cc in range(CCH):
                nc.tensor.matmul(mu_ps[half][:], ones_r[:],
                                 hT[:, cc, 512 * half : 512 * (half + 1)],
                                 start=(cc == 0), stop=(cc == CCH - 1))
                nc.tensor.matmul(sq_ps[half][:], ones_b[:],
                                 hsq[:, cc, 512 * half : 512 * (half + 1)],
                                 start=(cc == 0), stop=(cc == CCH - 1))
            nc.scalar.activation(mean_s[:, 512 * half : 512 * (half + 1)], mu_ps[half][:],
                                 AF.Copy, scale=1.0 / C)
            nc.scalar.activation(sqm_s[:, 512 * half : 512 * (half + 1)], sq_ps[half][:],
                                 AF.Copy, scale=1.0 / C)
        nc.vector.tensor_mul(m2_s[:], mean_s[:], mean_s[:])
        nc.vector.tensor_sub(sqm_s[:], sqm_s[:], m2_s[:])
        nc.scalar.activation(sqm_s[:], sqm_s[:], AF.Sqrt, bias=EPS)
        nc.vector.reciprocal(m2_s[:], sqm_s[:])
        nc.gpsimd.partition_broadcast(meanb[:], mean_s[:], channels=128)
        nc.gpsimd.partition_broadcast(istdb[:], m2_s[:], channels=128)
        for cc in range(CCH):
            tmp = pD2b.tile([128, N], F32, tag="tmp")
            nc.vector.tensor_sub(tmp[:], hT[:, cc, :], meanb[:])
            nc.vector.tensor_mul(tmp[:], tmp[:], istdb[:])
            nc.vector.tensor_scalar(hnT[:, cc, :], tmp[:], pw2[:, cc : cc + 1],
                                    pb2[:, cc : cc + 1], OP.mult, OP.add)

        # ================= MLP + residual ==================================
        Q = N // NQ
        for q in range(NQ):
            h1 = pD2.tile([128, HCH, N // NQ], BF16, tag="hsq_h1", name=f"h1_{q}")
            for hc in range(HCH):
                ps = pmm.tile([128, Q], F32, tag="mm")
                for cc in range(CCH):
                    nc.tensor.matmul(ps[:], f1w[:, cc, 128 * hc : 128 * (hc + 1)],
                                     hnT[:, cc, Q * q : Q * (q + 1)],
                                     start=(cc == 0), stop=(cc == CCH - 1))
                nc.scalar.activation(h1[:, hc, :], ps[:], AF.Gelu, bias=f1b[:, hc : hc + 1])
            for cc in range(CCH):
                ps = pacc.tile([128, Q], F32, tag="acc")
                for hc in range(HCH):
                    nc.tensor.matmul(ps[:], f2w[:, hc, 128 * cc : 128 * (cc + 1)], h1[:, hc, :],
                                     start=(hc == 0), stop=(hc == HCH - 1))
                nc.vector.tensor_add(outT[:, cc, Q * q : Q * (q + 1)], ps[:],
                                     hT[:, cc, Q * q : Q * (q + 1)])
                nc.vector.tensor_scalar(outT[:, cc, Q * q : Q * (q + 1)],
                                        outT[:, cc, Q * q : Q * (q + 1)],
                                        f2b[:, cc : cc + 1], None, OP.add)

        # ================= final transpose + store =========================
        for ncx in range(NCH):
            onat = pD2b.tile([128, C], F32, tag="onat")
            for cc in range(CCH):
                tp = pmm.tile([128, 128], F32R, tag="mm")
                nc.tensor.transpose(tp[:], outT[:, cc, 128 * ncx : 128 * (ncx + 1)], ident_r[:])
                nc.scalar.activation(onat[:, 128 * cc : 128 * (cc + 1)], tp[:], AF.Copy)
            nc.sync.dma_start(out=out_p[128 * ncx : 128 * (ncx + 1), :], in_=onat[:])

        pD2b.release()
        pD2.release()
        pD1.release()
        pC2.release()
        pt.release()
        p0.release()
        pacc.release()
        pmm.release()

    nc.finalize()
    return nc


_NC_CACHE = {}


def _get_nc():
    if "nc" not in _NC_CACHE:
        _NC_CACHE["nc"] = build()
    return _NC_CACHE["nc"]


def _prep_shared(inputs):
    """Host-side marshalling of weights into the exact SBUF layouts."""
    qkv = _f32(inputs["qkv_w"]).reshape(3, H, D, C)
    qk = np.concatenate([qkv[0] * SCALE, qkv[1]], axis=0)        # [16, 96, 768]
    qkw_t = qk.transpose(2, 0, 1).reshape(CCH, 128, 2 * H, D).transpose(1, 0, 2, 3)
    vw_t = qkv[2].reshape(H * D, C).T.reshape(CCH, 128, H * D).transpose(1, 0, 2)
    gwm = np.concatenate([_f32(inputs["wg_w"]), _f32(inputs["wg0_w"]),
                          _f32(inputs["wg1_w"])], axis=0)        # [10, 768]
    gw_t = gwm.T.reshape(CCH, 128, 10).transpose(1, 0, 2)
    projw_t = _f32(inputs["proj_w"]).reshape(C, H, D).transpose(2, 1, 0)  # [96, 8, 768]
    fc1w_t = _f32(inputs["fc1_w"]).T.reshape(CCH, 128, HID).transpose(1, 0, 2)
    fc2w_t = _f32(inputs["fc2_w"]).T.reshape(HCH, 128, C).transpose(1, 0, 2)

    def per_part(v, chunks):
        return _f32(v).reshape(chunks, 128).T.copy()

    return {
        "qkw_t": _bf(qkw_t), "vw_t": _bf(vw_t), "gw_t": _bf(gw_t),
        "projw_t": _bf(projw_t), "fc1w_t": _bf(fc1w_t), "fc2w_t": _bf(fc2w_t),
        "pw1": per_part(inputs["norm1_w"], CCH), "pb1": per_part(inputs["norm1_b"], CCH),
        "pw2": per_part(inputs["norm2_w"], CCH), "pb2": per_part(inputs["norm2_b"], CCH),
        "projb2": per_part(2.0 * _f32(inputs["proj_b"]), CCH),
        "fc1b": per_part(inputs["fc1_b"], HCH), "fc2b": per_part(inputs["fc2_b"], CCH),
        "ones_row": np.ones((1, H, N), dtype=np.float32),
    }


def make_in_maps(inputs):
    shared = _prep_shared(inputs)
    x = _f32(inputs["x"])
    in_maps = []
    for b in range(B):
        m = dict(shared)
        m["x"] = x[b].reshape(NCH, 128, C).transpose(1, 0, 2).copy()
        in_maps.append(m)
    return in_maps


def kernel(**inputs):
    from concourse.bass_utils import run_bass_kernel_spmd

    nc = _get_nc()
    in_maps = make_in_maps(inputs)
    res = run_bass_kernel_spmd(nc, in_maps, core_ids=list(range(B))).results
    out = np.stack([r["out"] for r in res])
    attn = np.stack([r["attn"] for r in res])
    return out, attn


# revision 15
# speedup vs baseline: 1.0641x; 1.0641x over previous
"""Trainium2 Bass kernel for nn_AACls_87282325389858 (moe_routing).

Data-parallel over batch B=8 across 8 NeuronCores (one batch element per
core, no collectives).  Per-core network:

  xn  = LN1(x)                                      [1024, 768]
  g   = concat-softmaxes(xn @ {wg,wg0,wg1}^T)  ->  per-head gates mg [1024, 8]
        (top_k(gates, 6) over 6 routed gates is the identity -> mask == 1)
  qkv = xn @ qkv_w^T ; per head: attn = softmax(q k^T / sqrt(96))  (output!)
  o_h = (mg_h / rowsum_h) * exp(scores_h) @ v_h   -- gate folded into an
        augmented contraction row:  exp(s + log(mg/rowsum))
  a   = concat_h(o_h) @ proj_w^T + proj_b ;  h = 2*a
  out = h + MLP(LN2(h))          ;  MLP = gelu(. @ fc1^T + b) @ fc2^T + b

Returns (out [8,1024,768] f32, attn [8,8,1024,1024] f32) matching the
reference tuple.
"""
import sys

sys.path.insert(0, "/opt/trn_rl_repo")

import numpy as np
import ml_dtypes

import concourse.bass as bass
import concourse.mybir as mybir
from concourse import bacc
from concourse.tile import TileContext
from concourse.masks import make_identity

F32 = mybir.dt.float32
F32R = mybir.dt.float32r
BF16 = mybir.dt.bfloat16
AF = mybir.ActivationFunctionType
OP = mybir.AluOpType
AX = mybir.AxisListType

B, N, C, H, D, HID = 8, 1024, 768, 8, 96, 3072
NCH = 8          # token chunks of 128
CCH = 6          # feature chunks of 128 (C = 768)
HCH = 24         # hidden chunks of 128 (HID = 3072)
SCALE = float(D) ** -0.5
EPS = 1e-5
NQ = 4           # n-quarters for the MLP (256 tokens each)

BF = ml_dtypes.bfloat16


def _bf(a):
    return np.ascontiguousarray(a).astype(BF)


def _f32(a):
    return np.ascontiguousarray(np.asarray(a, dtype=np.float32))


def build():
    nc = bacc.Bacc()

    # ---------------- parameters (per-core shapes) ----------------
    x_p = nc.declare_dram_parameter("x", [128, NCH, C], F32, isOutput=False)
    qkw_p = nc.declare_dram_parameter("qkw_t", [128, CCH, 2 * H, D], BF16, isOutput=False)
    vw_p = nc.declare_dram_parameter("vw_t", [128, CCH, H * D], BF16, isOutput=False)
    gw_p = nc.declare_dram_parameter("gw_t", [128, CCH, 10], BF16, isOutput=False)
    pjw_p = nc.declare_dram_parameter("projw_t", [D, H, C], BF16, isOutput=False)
    f1w_p = nc.declare_dram_parameter("fc1w_t", [128, CCH, HID], BF16, isOutput=False)
    f2w_p = nc.declare_dram_parameter("fc2w_t", [128, HCH, C], BF16, isOutput=False)
    pw1_p = nc.declare_dram_parameter("pw1", [128, CCH], F32, isOutput=False)
    pb1_p = nc.declare_dram_parameter("pb1", [128, CCH], F32, isOutput=False)
    pw2_p = nc.declare_dram_parameter("pw2", [128, CCH], F32, isOutput=False)
    pb2_p = nc.declare_dram_parameter("pb2", [128, CCH], F32, isOutput=False)
    pjb_p = nc.declare_dram_parameter("projb2", [128, CCH], F32, isOutput=False)
    f1b_p = nc.declare_dram_parameter("fc1b", [128, HCH], F32, isOutput=False)
    f2b_p = nc.declare_dram_parameter("fc2b", [128, CCH], F32, isOutput=False)
    ones_p = nc.declare_dram_parameter("ones_row", [1, H, N], F32R, isOutput=False)

    out_p = nc.declare_dram_parameter("out", [N, C], F32, isOutput=True)
    attn_p = nc.declare_dram_parameter("attn", [H, N, N], BF16, isOutput=True)

    with TileContext(nc) as tc:
        p0 = tc.alloc_tile_pool(name="const", bufs=1)
        pt = tc.alloc_tile_pool(name="tiny", bufs=6)
        pmm = tc.alloc_tile_pool(name="pmm", bufs=4, space="PSUM")
        pacc = tc.alloc_tile_pool(name="pacc", bufs=4, space="PSUM")

        # ---------------- constants ----------------
        for cv in (EPS, 1.0 / C):
            ct = nc.alloc_sbuf_tensor(f"constap-{cv}", [128, 1], F32)
            nc.gpsimd.memset(ct.ap(), cv)
            nc.const_aps.aps[(F32, cv)] = ct.ap()

        ident_f = p0.tile([128, 128], F32, tag="identf")
        make_identity(nc, ident_f)
        ident_r = p0.tile([128, 128], F32R, tag="identr")
        nc.vector.tensor_copy(ident_r[:], ident_f[:])
        ident_b = p0.tile([128, 128], BF16, tag="identb")
        nc.vector.tensor_copy(ident_b[:], ident_f[:])
        ones_f = p0.tile([128, 1], F32, tag="onesf")
        nc.vector.memset(ones_f[:], 1.0)
        ones_r = p0.tile([128, 1], F32R, tag="onesr")
        nc.vector.tensor_copy(ones_r[:], ones_f[:])
        ones_b = p0.tile([128, 1], BF16, tag="onesb")
        nc.vector.tensor_copy(ones_b[:], ones_f[:])

        pw1 = p0.tile([128, CCH], F32, tag="pw1")
        pb1 = p0.tile([128, CCH], F32, tag="pb1")
        pw2 = p0.tile([128, CCH], F32, tag="pw2")
        pb2 = p0.tile([128, CCH], F32, tag="pb2")
        pjb = p0.tile([128, CCH], F32, tag="pjb")
        f1b = p0.tile([128, HCH], F32, tag="f1b")
        f2b = p0.tile([128, CCH], F32, tag="f2b")
        for t, p in ((pw1, pw1_p), (pb1, pb1_p), (pw2, pw2_p), (pb2, pb2_p),
                     (pjb, pjb_p), (f1b, f1b_p), (f2b, f2b_p)):
            nc.sync.dma_start(out=t[:], in_=p[:])
        gw = p0.tile([128, CCH, 10], BF16, tag="gw")
        nc.sync.dma_start(out=gw[:], in_=gw_p[:])

        # gate / softmax working state (token-partition layout)
        musum = p0.tile([128, NCH], F32, tag="musum")
        sqsum = p0.tile([128, NCH], F32, tag="sqsum")
        mean_g = p0.tile([128, NCH], F32, tag="meang")
        var_g = p0.tile([128, NCH], F32, tag="varg")
        istd_g = p0.tile([128, NCH], F32, tag="istdg")
        ge = p0.tile([128, NCH, 10], F32, tag="ge")
        glg = p0.tile([128, NCH, 10], F32, tag="glg")
        gsum = p0.tile([128, 3 * NCH], F32, tag="gsum")
        ginv = p0.tile([128, 3 * NCH], F32, tag="ginv")
        mg = p0.tile([128, NCH, H], F32, tag="mg")
        mgr = mg[:].rearrange("p a b -> p (a b)")
        rs2 = p0.tile([128, NCH, H, 2], F32, tag="rs2")
        rs = p0.tile([128, NCH, H], F32, tag="rs")
        invr = p0.tile([128, NCH * H], F32, tag="invr")
        fct = p0.tile([128, NCH * H], F32, tag="fct")
        lgf = p0.tile([128, NCH * H], F32R, tag="lgf")

        # ================= phase A: LN1 + transposes + QKV + gates =========
        pA = tc.alloc_tile_pool(name="pA", bufs=1, side="right")
        pB1 = tc.alloc_tile_pool(name="pB1", bufs=1)
        x_sb = pA.tile([128, NCH, C], F32, tag="x")
        for ncx in range(NCH):
            nc.sync.dma_start(out=x_sb[:, ncx, :], in_=x_p[:, ncx, :])
        xn_bf = pA.tile([128, NCH, C], BF16, tag="xn")
        xnT = pB1.tile([128, CCH, N], BF16, tag="xnT")
        qkw = pB1.tile([128, CCH, 2 * H, D], BF16, tag="qkw")
        vw = pA.tile([128, CCH, H * D], BF16, tag="vw")
        nc.sync.dma_start(out=qkw[:], in_=qkw_p[:])
        nc.sync.dma_start(out=vw[:], in_=vw_p[:])

        # LN1: stats per chunk, one batched sqrt, then normalize + transpose
        for ncx in range(NCH):
            nc.vector.reduce_sum(musum[:, ncx : ncx + 1], x_sb[:, ncx, :], axis=AX.X)
            scr = pA.tile([128, C], F32, tag="sqscr")
            nc.scalar.activation(scr[:], x_sb[:, ncx, :], AF.Square,
                                 accum_out=sqsum[:, ncx : ncx + 1])
        nc.vector.tensor_scalar(mean_g[:], musum[:], 1.0 / C, None, OP.mult)
        nc.vector.tensor_scalar(var_g[:], sqsum[:], 1.0 / C, None, OP.mult)
        m2t = pt.tile([128, NCH], F32, tag="m2")
        nc.vector.tensor_mul(m2t[:], mean_g[:], mean_g[:])
        nc.vector.tensor_sub(var_g[:], var_g[:], m2t[:])
        nc.scalar.activation(var_g[:], var_g[:], AF.Sqrt, bias=EPS)
        nc.vector.reciprocal(istd_g[:], var_g[:])
        for ncx in range(NCH):
            nc.vector.tensor_scalar(xn_bf[:, ncx, :], x_sb[:, ncx, :],
                                    mean_g[:, ncx : ncx + 1], istd_g[:, ncx : ncx + 1],
                                    OP.subtract, OP.mult)
            for cc in range(CCH):
                tp = pmm.tile([128, 128], BF16, tag="mm")
                nc.tensor.transpose(tp[:], xn_bf[:, ncx, 128 * cc : 128 * (cc + 1)], ident_b[:])
                nc.vector.tensor_scalar(xnT[:, cc, 128 * ncx : 128 * (ncx + 1)], tp[:],
                                        pw1[:, cc : cc + 1], pb1[:, cc : cc + 1],
                                        OP.mult, OP.add)

        # ---- phase B tensors (left side; allocated before A dies) ----
        qT = pB1.tile([D + 1, H, N], F32R, tag="qT")
        kT = pB1.tile([D + 1, H, N], F32R, tag="kT")
        v_sb = pB1.tile([128, NCH, H, D], BF16, tag="v")

        # v + gates per token-chunk (start while later LN1 chunks still run)
        for ncx in range(NCH):
            for half, (lo, w) in enumerate(((0, 512), (512, 256))):
                ps = pacc.tile([128, 512], F32, tag="acc", name=f"vps{ncx}_{half}")
                for cc in range(CCH):
                    nc.tensor.matmul(ps[:, :w], xnT[:, cc, 128 * ncx : 128 * (ncx + 1)],
                                     vw[:, cc, lo : lo + w],
                                     start=(cc == 0), stop=(cc == CCH - 1))
                nc.vector.tensor_copy(v_sb[:, ncx, :, :].rearrange("p h d -> p (h d)")[:, lo : lo + w],
                                      ps[:, :w])
            psg = pmm.tile([128, 10], F32, tag="mm", name=f"gps{ncx}")
            for cc in range(CCH):
                nc.tensor.matmul(psg[:], xnT[:, cc, 128 * ncx : 128 * (ncx + 1)], gw[:, cc, :],
                                 start=(cc == 0), stop=(cc == CCH - 1))
            nc.vector.tensor_copy(glg[:, ncx, :], psg[:])
        # ones row of kT (from host constant)
        nc.sync.dma_start(out=kT[D : D + 1, :, :], in_=ones_p[:])


        # gate softmaxes (deferred exp; one table set shared with attention)
        for ncx in range(NCH):
            nc.scalar.activation(ge[:, ncx, :], glg[:, ncx, :], AF.Exp)
            nc.vector.reduce_sum(gsum[:, 3 * ncx : 3 * ncx + 1], ge[:, ncx, 0:6], axis=AX.X)
            nc.vector.reduce_sum(gsum[:, 3 * ncx + 1 : 3 * ncx + 2], ge[:, ncx, 6:8], axis=AX.X)
            nc.vector.reduce_sum(gsum[:, 3 * ncx + 2 : 3 * ncx + 3], ge[:, ncx, 8:10], axis=AX.X)
        nc.vector.reciprocal(ginv[:], gsum[:])

        # mg[:, :, 0:2] = 4 * e8..9 * e6 / (s0 * s1);  mg[:, :, 2:8] = 12 * e0..5 * e7 / (s0 * sr)
        for ncx in range(NCH):
            ta = pt.tile([128, 1], F32, tag="ta")
            tb = pt.tile([128, 1], F32, tag="tb")
            nc.vector.tensor_mul(ta[:], ge[:, ncx, 6:7], ginv[:, 3 * ncx + 1 : 3 * ncx + 2])
            nc.vector.tensor_mul(ta[:], ta[:], ginv[:, 3 * ncx + 2 : 3 * ncx + 3])
            nc.vector.tensor_scalar(ta[:], ta[:], 4.0, None, OP.mult)
            nc.vector.tensor_mul(tb[:], ge[:, ncx, 7:8], ginv[:, 3 * ncx + 1 : 3 * ncx + 2])
            nc.vector.tensor_mul(tb[:], tb[:], ginv[:, 3 * ncx : 3 * ncx + 1])
            nc.vector.tensor_scalar(tb[:], tb[:], 12.0, None, OP.mult)
            nc.vector.tensor_scalar(mg[:, ncx, 0:2], ge[:, ncx, 8:10], ta[:], None, OP.mult)
            nc.vector.tensor_scalar(mg[:, ncx, 2:8], ge[:, ncx, 0:6], tb[:], None, OP.mult)

        pA.release()
        pB2 = tc.alloc_tile_pool(name="pB2", bufs=2)
        pLg = tc.alloc_tile_pool(name="pLg", bufs=1)

        # ================= phase B: attention heads ========================
        pC2 = tc.alloc_tile_pool(name="pC2", bufs=1, side="right")
        pC1 = tc.alloc_tile_pool(name="pC1", bufs=1, side="right")
        hT = pC2.tile([128, CCH, N], F32R, tag="hT")
        o_gT = pC1.tile([D, H, N], BF16, tag="ogT")
        pjw = pC1.tile([D, H, C], BF16, tag="pjw")
        nc.sync.dma_start(out=pjw[:], in_=pjw_p[:])

        def natural(h):
            """scores -> exp -> rowsums -> normalized attn -> HBM"""
            for ncx in range(NCH):
                exp_sb = pB2.tile([128, N], F32, tag="exp")
                for half in range(2):
                    ps = pmm.tile([128, 512], F32, tag="mm")
                    nc.tensor.matmul(ps[:], qT[0:D, h, 128 * ncx : 128 * (ncx + 1)],
                                     kT[0:D, h, 512 * half : 512 * (half + 1)],
                                     start=True, stop=True)
                    nc.scalar.activation(exp_sb[:, 512 * half : 512 * (half + 1)], ps[:],
                                         AF.Exp, accum_out=rs2[:, ncx, h, half : half + 1])
                nc.vector.tensor_add(rs[:, ncx, h : h + 1], rs2[:, ncx, h, 0:1],
                                     rs2[:, ncx, h, 1:2])
                ti = pt.tile([128, 1], F32, tag="ti")
                nc.vector.reciprocal(ti[:], rs[:, ncx, h : h + 1])
                attn_bf = pB2.tile([128, N], BF16, tag="attnbf")
                nc.vector.tensor_scalar(attn_bf[:], exp_sb[:], ti[:], None, OP.mult)
                nc.sync.dma_start(out=attn_p[h, 128 * ncx : 128 * (ncx + 1), :], in_=attn_bf[:])

        def qkproj(h):
            for i in (h, h + H):
                dst = qT if i < H else kT
                for half in range(2):
                    ps = pacc.tile([D, 512], F32, tag="acc", name=f"qk{i}_{half}")
                    for cc in range(CCH):
                        nc.tensor.matmul(ps[:], qkw[:, cc, i, :],
                                         xnT[:, cc, 512 * half : 512 * (half + 1)],
                                         start=(cc == 0), stop=(cc == CCH - 1))
                    nc.vector.tensor_copy(dst[0:D, h, 512 * half : 512 * (half + 1)], ps[:])

        def logfactor_all():
            for h in range(H):
                nc.vector.reciprocal(invr[:, NCH * h : NCH * (h + 1)], rs[:, :, h])
                nc.vector.tensor_mul(fct[:, NCH * h : NCH * (h + 1)], mg[:, :, h],
                                     invr[:, NCH * h : NCH * (h + 1)])
            nc.scalar.activation(lgf[:], fct[:], AF.Ln)

        def transposed(h):
            """scoresT + gate row -> exp -> o_gT = v^T @ expT"""
            lgT = pLg.tile([1, N], F32R, tag="lgT")
            for half in range(2):
                lps = pmm.tile([1, 512], F32R, tag="mm")
                for j in range(4):
                    ncx = 4 * half + j
                    nc.tensor.transpose(lps[:, 128 * j : 128 * (j + 1)],
                                        lgf[:, NCH * h + ncx : NCH * h + ncx + 1], ident_r[:])
                nc.vector.tensor_copy(lgT[:, 512 * half : 512 * (half + 1)], lps[:])
            nc.sync.dma_start(out=qT[D : D + 1, h, :], in_=lgT[:])

            oT = [pacc.tile([D, 512], F32, tag="acc", name=f"oT{h}_{j}") for j in range(2)]
            for mc in range(NCH):
                expT = pB2.tile([128, N], BF16, tag="expT")
                for half in range(2):
                    ps = pmm.tile([128, 512], F32, tag="mm")
                    nc.tensor.matmul(ps[:], kT[:, h, 128 * mc : 128 * (mc + 1)],
                                     qT[:, h, 512 * half : 512 * (half + 1)],
                                     start=True, stop=True)
                    nc.scalar.activation(expT[:, 512 * half : 512 * (half + 1)], ps[:], AF.Exp)
                for half in range(2):
                    nc.tensor.matmul(oT[half][:], v_sb[:, mc, h, :],
                                     expT[:, 512 * half : 512 * (half + 1)],
                                     start=(mc == 0), stop=(mc == NCH - 1))
            for half in range(2):
                nc.vector.tensor_copy(o_gT[:, h, 512 * half : 512 * (half + 1)], oT[half][:])

        qkproj(0)
        for h in range(H):
            if h + 1 < H:
                qkproj(h + 1)
            natural(h)
        logfactor_all()
        for h in range(H):
            transposed(h)

        pLg.release()
        pB2.release()
        pB1.release()

        # ================= proj + residual (hT = 2*(a + proj_b)) ==========
        pD1 = tc.alloc_tile_pool(name="pD1", bufs=1)
        f1w = pD1.tile([128, CCH, HID], BF16, tag="f1w")
        f2w = pD1.tile([128, HCH, C], BF16, tag="f2w")
        nc.sync.dma_start(out=f1w[:], in_=f1w_p[:])
        nc.sync.dma_start(out=f2w[:], in_=f2w_p[:])

        for cc in range(CCH):
            for half in range(2):
                ps = pacc.tile([128, 512], F32, tag="acc")
                for h in range(H):
                    nc.tensor.matmul(ps[:], pjw[:, h, 128 * cc : 128 * (cc + 1)],
                                     o_gT[:, h, 512 * half : 512 * (half + 1)],
                                     start=(h == 0), stop=(h == H - 1))
                nc.vector.tensor_scalar(hT[:, cc, 512 * half : 512 * (half + 1)], ps[:],
                                        2.0, pjb[:, cc : cc + 1], OP.mult, OP.add)
        pC1.release()

        # ================= LN2 ============================================
        pD2 = tc.alloc_tile_pool(name="pD2", bufs=1)
        pD2b = tc.alloc_tile_pool(name="pD2b", bufs=2)
        hsq = pD2.tile([128, CCH, N], BF16, tag="hsq_h1")  # 12KB/part, slot shared with h1
        hnT = pD2.tile([128, CCH, N], BF16, tag="hnT")
        
        outT = pD2.tile([128, CCH, N], F32R, tag="outT")
        meanb = pD2.tile([128, N], F32, tag="meanb")
        istdb = pD2.tile([128, N], F32, tag="istdb")
        mean_s = pD2.tile([1, N], F32, tag="means")
        sqm_s = pD2.tile([1, N], F32, tag="sqms")
        m2_s = pD2.tile([1, N], F32, tag="m2s")

        for cc in range(CCH):
            nc.scalar.activation(hsq[:, cc, :], hT[:, cc, :], AF.Square)
        mu_ps = [pmm.tile([1, 512], F32, tag="mm", name=f"mups{j}") for j in range(2)]
        sq_ps = [pmm.tile([1, 512], F32, tag="mm", name=f"sqps{j}") for j in range(2)]
        for half in range(2):
            for cc in range(CCH):
                nc.tensor.matmul(mu_ps[half][:], ones_r[:],
                                 hT[:, cc, 512 * half : 512 * (half + 1)],
                                 start=(cc == 0), stop=(cc == CCH - 1))
                nc.tensor.matmul(sq_ps[half][:], ones_b[:],
                                 hsq[:, cc, 512 * half : 512 * (half + 1)],
                                 start=(cc == 0), stop=(cc == CCH - 1))
            nc.vector.tensor_scalar(mean_s[:, 512 * half : 512 * (half + 1)], mu_ps[half][:],
                                    1.0 / C, None, OP.mult)
            nc.vector.tensor_scalar(sqm_s[:, 512 * half : 512 * (half + 1)], sq_ps[half][:],
                                    1.0 / C, None, OP.mult)
        nc.vector.tensor_mul(m2_s[:], mean_s[:], mean_s[:])
        nc.vector.tensor_sub(sqm_s[:], sqm_s[:], m2_s[:])
        nc.scalar.activation(sqm_s[:], sqm_s[:], AF.Sqrt, bias=EPS)
        nc.vector.reciprocal(m2_s[:], sqm_s[:])
        nc.gpsimd.partition_broadcast(meanb[:], mean_s[:], channels=128)
        nc.gpsimd.partition_broadcast(istdb[:], m2_s[:], channels=128)
        for cc in range(CCH):
            tmp = pD2b.tile([128, N], F32, tag="tmp")
            nc.vector.tensor_sub(tmp[:], hT[:, cc, :], meanb[:])
            nc.vector.tensor_mul(tmp[:], tmp[:], istdb[:])
            nc.vector.tensor_scalar(hnT[:, cc, :], tmp[:], pw2[:, cc : cc + 1],
                                    pb2[:, cc : cc + 1], OP.mult, OP.add)

        # ================= MLP + residual ==================================
        Q = N // NQ
        for q in range(NQ):
            h1 = pD2.tile([128, HCH, N // NQ], BF16, tag="hsq_h1", name=f"h1_{q}")
            for hc in range(HCH):
                ps = pmm.tile([128, Q], F32, tag="mm")
                for cc in range(CCH):
                    nc.tensor.matmul(ps[:], f1w[:, cc, 128 * hc : 128 * (hc + 1)],
                                     hnT[:, cc, Q * q : Q * (q + 1)],
                                     start=(cc == 0), stop=(cc == CCH - 1))
                nc.scalar.activation(h1[:, hc, :], ps[:], AF.Gelu, bias=f1b[:, hc : hc + 1])
            for cc in range(CCH):
                ps = pacc.tile([128, Q], F32, tag="acc")
                for hc in range(HCH):
                    nc.tensor.matmul(ps[:], f2w[:, hc, 128 * cc : 128 * (cc + 1)], h1[:, hc, :],
                                     start=(hc == 0), stop=(hc == HCH - 1))
                nc.vector.tensor_add(outT[:, cc, Q * q : Q * (q + 1)], ps[:],
                                     hT[:, cc, Q * q : Q * (q + 1)])
                nc.vector.tensor_scalar(outT[:, cc, Q * q : Q * (q + 1)],
                                        outT[:, cc, Q * q : Q * (q + 1)],
                                        f2b[:, cc : cc + 1], None, OP.add)
            # final transpose + store for this quarter's token chunks
            for ncx in range(2 * q, 2 * (q + 1)):
                onat = pD2b.tile([128, C], F32, tag="onat", name=f"onat{ncx}")
                for cc in range(CCH):
                    tp = pmm.tile([128, 128], F32R, tag="mm", name=f"otp{ncx}_{cc}")
                    nc.tensor.transpose(tp[:], outT[:, cc, 128 * ncx : 128 * (ncx + 1)], ident_r[:])
                    nc.vector.tensor_copy(onat[:, 128 * cc : 128 * (cc + 1)], tp[:])
                nc.sync.dma_start(out=out_p[128 * ncx : 128 * (ncx + 1), :], in_=onat[:])

        pD2b.release()
        pD2.release()
        pD1.release()
        pC2.release()
        pt.release()
        p0.release()
        pacc.release()
        pmm.release()

    nc.finalize()
    return nc


_NC_CACHE = {}


def _get_nc():
    if "nc" not in _NC_CACHE:
        _NC_CACHE["nc"] = build()
    return _NC_CACHE["nc"]


def _prep_shared(inputs):
    """Host-side marshalling of weights into the exact SBUF layouts."""
    qkv = _f32(inputs["qkv_w"]).reshape(3, H, D, C)
    qk = np.concatenate([qkv[0] * SCALE, qkv[1]], axis=0)        # [16, 96, 768]
    qkw_t = qk.transpose(2, 0, 1).reshape(CCH, 128, 2 * H, D).transpose(1, 0, 2, 3)
    vw_t = qkv[2].reshape(H * D, C).T.reshape(CCH, 128, H * D).transpose(1, 0, 2)
    gwm = np.concatenate([_f32(inputs["wg_w"]), _f32(inputs["wg0_w"]),
                          _f32(inputs["wg1_w"])], axis=0)        # [10, 768]
    gw_t = gwm.T.reshape(CCH, 128, 10).transpose(1, 0, 2)
    projw_t = _f32(inputs["proj_w"]).reshape(C, H, D).transpose(2, 1, 0)  # [96, 8, 768]
    fc1w_t = _f32(inputs["fc1_w"]).T.reshape(CCH, 128, HID).transpose(1, 0, 2)
    fc2w_t = _f32(inputs["fc2_w"]).T.reshape(HCH, 128, C).transpose(1, 0, 2)

    def per_part(v, chunks):
        return _f32(v).reshape(chunks, 128).T.copy()

    return {
        "qkw_t": _bf(qkw_t), "vw_t": _bf(vw_t), "gw_t": _bf(gw_t),
        "projw_t": _bf(projw_t), "fc1w_t": _bf(fc1w_t), "fc2w_t": _bf(fc2w_t),
        "pw1": per_part(inputs["norm1_w"], CCH), "pb1": per_part(inputs["norm1_b"], CCH),
        "pw2": per_part(inputs["norm2_w"], CCH), "pb2": per_part(inputs["norm2_b"], CCH),
        "projb2": per_part(2.0 * _f32(inputs["proj_b"]), CCH),
        "fc1b": per_part(inputs["fc1_b"], HCH), "fc2b": per_part(inputs["fc2_b"], CCH),
        "ones_row": np.ones((1, H, N), dtype=np.float32),
    }


def make_in_maps(inputs):
    shared = _prep_shared(inputs)
    x = _f32(inputs["x"])
    in_maps = []
    for b in range(B):
        m = dict(shared)
        m["x"] = x[b].reshape(NCH, 128, C).transpose(1, 0, 2).copy()
        in_maps.append(m)
    return in_maps


def kernel(**inputs):
    from concourse.bass_utils import run_bass_kernel_spmd

    nc = _get_nc()
    in_maps = make_in_maps(inputs)
    res = run_bass_kernel_spmd(nc, in_maps, core_ids=list(range(B))).results
    out = np.stack([r["out"] for r in res])
    attn = np.stack([r["attn"] for r in res])
    return out, attn
